# revision 11
# baseline (speedup 1.0000x reference)
"""DSAttention Trainium2 kernel (v4).

Reference computation (per batch b, head h):
    S[q,s]  = (Q[q]·K[s]) * tau[b] + delta[b,s]
    S      += causal mask (s > q -> -inf)
    A       = softmax(S / sqrt(E), axis=s)
    O[q,:]  = sum_s A[q,s] * V[s,:]

Shapes: B=2, L=2048, H=16, E=64 -> 32 (b,h) pairs, 4 per NeuronCore x 8 cores.

v4 design. Engine-work floors per core (measured v2): PE ~55us of matmul
column streaming (QK 17408 + AV 17408 cols/head at 1 col/cycle; tiling
cannot beat this - the ifmap XBUS serializes any second stream), ACT+DVE
~58us of exp split between them, GpSimd ~30us of causal masks. v2 hit 108us
because the pipeline only kept the busiest engine 75% fed: the 2-deep
[128,1024] PSUM ring coupled QK(n+2) to the full 1024-wide exp(n), and
every phase/head boundary flushed the AV lag queue. v4 restructures the
schedule around the same math:
  - S PSUM ring: 4 x [128,512] tiles (4 banks). Each QK piece (<=512 cols)
    gets its own tile; exp runs per piece, so the producer-consumer loop
    QK(p) -> exp(p) -> QK(p+4) spans 4 pieces and stops binding the rate.
  - exp pieces alternate ACT (native Exp, scale=ln2) / DVE (Schraudolph
    fp16 bit trick written through an int16-bitcast view of the fp16 a_sb
    tile), greedily targeting ~1/3 of columns on DVE.
  - One flat emission pipeline across all (head, phase, chunk): the AV lag
    queue (4 pieces) drains into the next phase's QK stream instead of
    flushing at boundaries; oT [65,1024] double-buffered per phase.
  - All 4 heads' qt/kt/vt DMA upfront (SBUF is big enough) - no head
    boundary load stalls.
  - Layout as v2: qt/kt [65, L] fp16 (rows 0-63 Q^T*(tau*log2e/8) / K^T,
    row 64 ones / delta*log2e/8), vt [128, 16*65] with a ones column, so
    the QK matmul emits t = log2(e^(score/8)) and AV row 64 the softmax
    denominator. Host divides + transposes.
"""

import sys

sys.path.insert(0, "/opt/trn_rl_repo")

import numpy as np

import concourse.bass as bass
import concourse.tile as tile
from concourse import bacc, mybir
from concourse.masks import make_upper_triangular

B, L, H, E = 2, 2048, 16, 64
NCORES = 8
HPC = (B * H) // NCORES  # heads per core = 4
NCH = L // 128  # s-chunks per head = 16
LOG2E = 1.4426950408889634
LN2 = 0.6931471805599453
FOLD = LOG2E / 8.0  # folds the 1/sqrt(E) softmax scale + base-2 conversion
F32 = mybir.dt.float32
F16 = mybir.dt.float16
I16 = mybir.dt.int16
EXP = mybir.ActivationFunctionType.Exp
MULT = mybir.AluOpType.mult
ADD = mybir.AluOpType.add

# Schraudolph constant for fp16 (exp bias 15, 10 mantissa bits):
# bitcast_f16(int16(t*2^10 + B10)) ~= 2^t, C tuned for mean relative error.
B10 = 15.0 * 1024.0 - 0.00725 * 1024.0
DVE_FRAC = 0.34  # target fraction of exp columns on DVE


def _body(tc, qT, kT, v1, out):
    nc = tc.nc
    from contextlib import ExitStack

    with ExitStack() as ctx:
        const = ctx.enter_context(tc.tile_pool(name="const", bufs=1))
        qk_pool = ctx.enter_context(tc.tile_pool(name="qk", bufs=HPC))
        v_pool = ctx.enter_context(tc.tile_pool(name="v", bufs=HPC))
        a_pool = ctx.enter_context(tc.tile_pool(name="a", bufs=4))
        o_pool = ctx.enter_context(tc.tile_pool(name="o", bufs=2))
        # PSUM budget (8 banks): 2x [128,1024] (4) + 2x [128,512] (2) S
        # tiles + 1x [65,1024] oT (2). Two-piece chunks draw from psA,
        # single-piece chunks from psB, giving a 4-tile S ring.
        psA_pool = ctx.enter_context(tc.tile_pool(name="psA", bufs=2, space="PSUM"))
        psB_pool = ctx.enter_context(tc.tile_pool(name="psB", bufs=2, space="PSUM"))
        po_pool = ctx.enter_context(tc.tile_pool(name="psO", bufs=1, space="PSUM"))

        trimask = const.tile([128, 128], F16, name="trimask")
        make_upper_triangular(nc, trimask[:], val=1.0, diag=True)

        # Load all heads upfront; DMA spreads over the whole kernel.
        qts, kts, vts = [], [], []
        for i in range(HPC):
            qt = qk_pool.tile([65, L], F16, tag=f"qt{i}", name=f"qt{i}")
            kt = qk_pool.tile([65, L], F16, tag=f"kt{i}", name=f"kt{i}")
            vt = v_pool.tile([128, NCH * 65], F16, tag=f"vt{i}", name=f"vt{i}")
            for hf in range(2):
                cs = slice(1024 * hf, 1024 * hf + 1024)
                nc.sync.dma_start(kt[:, cs], kT[i][:, cs])
                nc.sync.dma_start(qt[:, cs], qT[i][:, cs])
                vs = slice(8 * 65 * hf, 8 * 65 * hf + 8 * 65)
                nc.sync.dma_start(vt[:, vs], v1[i][:, vs])
            qts.append(qt)
            kts.append(kt)
            vts.append(vt)

        pend = []  # (i, phase, n, c0, w, a_sb, oT) AV pieces awaiting emission
        dve_cols = 0
        tot_cols = 0

        def emit_av(u):
            i, phase, n, c0, w, a_sb, oT = u
            qlo = 1024 * phase
            j = (c0 - qlo) // 512
            nc.tensor.matmul(
                oT[:, c0 - qlo : c0 - qlo + w],
                lhsT=vts[i][:, n * 65 : n * 65 + 65],
                rhs=a_sb[:, c0 - qlo : c0 - qlo + w],
                start=(n == 0),
                stop=(n == 8 * phase + 4 * j + 3),
            )

        for i in range(HPC):
            for phase in range(2):
                qlo = 1024 * phase
                qhi = qlo + 1024
                oT = po_pool.tile([65, 1024], F32, tag="oT", name=f"oT{i}_{phase}")

                for n in range(qhi // 128):
                    q0 = max(128 * n, qlo)
                    two_piece = q0 < qlo + 512
                    if two_piece:
                        pieces = [(q0, qlo + 512 - q0), (qlo + 512, 512)]
                        ps = psA_pool.tile(
                            [128, 1024], F32, tag="ps", name=f"ps{i}_{phase}_{n}"
                        )
                    else:
                        pieces = [(q0, qhi - q0)]
                        ps = psB_pool.tile(
                            [128, 512], F32, tag="ps", name=f"ps{i}_{phase}_{n}"
                        )
                    a_sb = a_pool.tile(
                        [128, 1024], F16, tag="a", name=f"a{i}_{phase}_{n}"
                    )
                    # QK pieces back to back: same kt weights, so the second
                    # piece's LDWEIGHTS is wait-free and overlaps.
                    for pi, (c0, w) in enumerate(pieces):
                        psl = (
                            slice(c0 - qlo, c0 - qlo + w)
                            if two_piece
                            else slice(0, w)
                        )
                        nc.tensor.matmul(
                            ps[:, psl],
                            lhsT=kts[i][:, 128 * n : 128 * n + 128],
                            rhs=qts[i][:, c0 : c0 + w],
                            start=True,
                            stop=True,
                        )
                    for pi, (c0, w) in enumerate(pieces):
                        psl = (
                            slice(c0 - qlo, c0 - qlo + w)
                            if two_piece
                            else slice(0, w)
                        )
                        asl = slice(c0 - qlo, c0 - qlo + w)
                        use_dve = dve_cols < DVE_FRAC * tot_cols
                        tot_cols += w
                        if use_dve:
                            dve_cols += w
                            nc.vector.tensor_scalar(
                                a_sb[:, asl].bitcast(I16),
                                ps[:, psl],
                                1024.0,
                                B10,
                                MULT,
                                ADD,
                            )
                        else:
                            nc.scalar.activation(
                                a_sb[:, asl], ps[:, psl], EXP, scale=LN2
                            )
                        if pi == 0 and 128 * n >= qlo:
                            nc.gpsimd.tensor_mul(
                                a_sb[:, q0 - qlo : q0 - qlo + 128],
                                a_sb[:, q0 - qlo : q0 - qlo + 128],
                                trimask[:],
                            )
                        pend.append((i, phase, n, c0, w, a_sb, oT))
                        if len(pend) > 4:
                            emit_av(pend.pop(0))
                # oT is single-buffered: flush this phase's AV tail and
                # emit the merge+store now. QK of the next phase proceeds
                # on the S ring while the merge drains.
                for u in pend:
                    emit_av(u)
                pend.clear()
                o_sb = o_pool.tile([65, 1024], F32, tag="osb", name=f"osb{i}_{phase}")
                nc.vector.tensor_copy(o_sb[:], oT[:])
                nc.sync.dma_start(out[i][:, qlo:qhi], o_sb[:])


_CACHED = None


def _build():
    global _CACHED
    if _CACHED is not None:
        return _CACHED
    nc = bacc.Bacc("TRN2", target_bir_lowering=False, debug=False)
    qT = nc.dram_tensor("qT", [HPC, 65, L], F16, kind="ExternalInput").ap()
    kT = nc.dram_tensor("kT", [HPC, 65, L], F16, kind="ExternalInput").ap()
    v1 = nc.dram_tensor("v1", [HPC, 128, NCH * 65], F16, kind="ExternalInput").ap()
    out = nc.dram_tensor("out", [HPC, 65, L], F32, kind="ExternalOutput").ap()
    with tile.TileContext(nc) as tc:
        _body(tc, qT, kT, v1, out)
    nc.compile()
    _CACHED = nc
    return nc


def _prep_in_maps(queries, keys, values, tau, delta):
    """Shard + relayout the full inputs into 8 per-core input dicts."""
    queries = np.asarray(queries, dtype=np.float32)
    keys = np.asarray(keys, dtype=np.float32)
    values = np.asarray(values, dtype=np.float32)
    tau = np.asarray(tau, dtype=np.float32)
    delta = np.asarray(delta, dtype=np.float32)

    in_maps = []
    for core in range(NCORES):
        qTs = np.zeros((HPC, 65, L), np.float16)
        kTs = np.zeros((HPC, 65, L), np.float16)
        v1s = np.empty((HPC, 128, NCH * 65), np.float16)
        for slot in range(HPC):
            g = core * HPC + slot
            b, h = divmod(g, H)
            qTs[slot, 0:64] = queries[b, :, h, :].T * (tau[b, 0] * FOLD)
            qTs[slot, 64, :] = 1.0
            kTs[slot, 0:64] = keys[b, :, h, :].T
            kTs[slot, 64, :] = delta[b, :] * FOLD
            v = values[b, :, h, :].reshape(NCH, 128, E).transpose(1, 0, 2)
            vv = np.concatenate([v, np.ones((128, NCH, 1), np.float32)], axis=2)
            v1s[slot] = vv.reshape(128, NCH * 65).astype(np.float16)
        in_maps.append({"qT": qTs, "kT": kTs, "v1": v1s})
    return in_maps


def _assemble(results):
    O = np.empty((B, L, H, E), np.float32)
    for core in range(NCORES):
        o = results[core]["out"]  # [HPC, 65, L]
        for slot in range(HPC):
            g = core * HPC + slot
            b, h = divmod(g, H)
            O[b, :, h, :] = (o[slot, 0:64, :] / o[slot, 64:65, :]).T
    return O


def run(inputs, trace=False, **kwargs):
    from concourse import bass_utils

    nc = _build()
    in_maps = _prep_in_maps(**inputs)
    res = bass_utils.run_bass_kernel_spmd(
        nc, in_maps, core_ids=list(range(NCORES)), trace=trace, **kwargs
    )
    return _assemble(res.results), res


def kernel(**inputs):
    return run(inputs, trace=False)[0]
